# revision 1
# baseline (speedup 1.0000x reference)
"""Causal multi-head attention (B=2, T=4096, D=1024, H=16, HD=64) on 8 trn2
NeuronCores.

Sharding: core c handles batch b = c//4 and head group g = c%4 (heads
4g..4g+3).  Each core computes qkv projection for its 4 heads, causal
flash-attention in transposed (S^T) layout, and a partial out-projection
(its 256 columns of the hidden dim).  Host sums the 4 partial outputs per
batch and adds the bias terms.

Math notes:
  - k-bias dropped on device (softmax-invariant: adds a per-query constant
    to every score row).
  - v-bias folded into the host epilogue: softmax rows sum to 1, so
    out += b_v exactly, hence y += b_v @ w_out (+ b_out) host-side.
  - softmax computed without max subtraction (scores are O(10) for this
    problem scale; exp stays in fp32 range).
  - softmax denominators come for free as a 65th ones-column in v.
dtypes: q/k path float32r (TF32-like, ~1e-4), P and v bf16, accum fp32.
"""

import numpy as np

import concourse.bass as bass
import concourse.mybir as mybir
import concourse.tile as tile
from concourse import bacc
from concourse.bass_utils import run_bass_kernel_spmd
from concourse.masks import make_upper_triangular

F32 = mybir.dt.float32
F32R = mybir.dt.float32r
BF16 = mybir.dt.bfloat16
AF = mybir.ActivationFunctionType

B, D, H, HD = 2, 1024, 16, 64
NHEADS = 4          # heads per core
SCALE = 1.0 / np.sqrt(HD)


def build(T=4096, reps=1, skip_attn=False):
    """Build the per-core Bass module. reps>1 wraps the compute in an
    on-device For_i loop (for wall-clock-difference timing)."""
    NJ = T // 512       # tq tiles of 512
    NT = T // 128       # t chunks of 128
    DC = D // 128       # d chunks of 128

    nc = bacc.Bacc("TRN2", target_bir_lowering=False, debug=False, num_devices=8)

    xt_d = nc.dram_tensor("xt", [D, T], F32R, kind="ExternalInput")
    wqk_d = nc.dram_tensor("wqk", [D, 512], F32R, kind="ExternalInput")
    wv_d = nc.dram_tensor("wv", [D, 256], F32R, kind="ExternalInput")
    bqk_d = nc.dram_tensor("bqk", [128, 4], F32, kind="ExternalInput")
    wout_d = nc.dram_tensor("wout", [64, 4, D], F32R, kind="ExternalInput")
    y_d = nc.dram_tensor("y", [T, D], F32, kind="ExternalOutput")
    nrm_d = nc.dram_tensor("nrm", [1, 8, T], F32)  # rows 0-3 sums, 4-7 recips

    with tile.TileContext(nc) as tc:
        with (
            tc.tile_pool(name="const", bufs=1) as cp,
            tc.tile_pool(name="persist", bufs=1) as pp,
        ):
            # ---------- constants / weights (outside the timing loop)
            tri32 = cp.tile([128, 128], F32, tag="tri32")
            make_upper_triangular(nc, tri32[:], val=1.0, diag=True)
            tri = cp.tile([128, 128], BF16, tag="tri")
            nc.vector.tensor_copy(tri[:], tri32[:])

            ones1 = cp.tile([1, 64], F32, tag="ones1")
            nc.vector.memset(ones1[:], 1.0)
            ones1r = cp.tile([1, 64], F32R, tag="ones1r")
            nc.vector.tensor_copy(ones1r[:], ones1[:])

            bqk_sb = cp.tile([128, 4], F32, tag="bqk")
            nc.sync.dma_start(bqk_sb[:], bqk_d[:])
            wqk_sb = cp.tile([128, DC, 512], F32R, tag="wqk")
            nc.sync.dma_start(wqk_sb[:], wqk_d.rearrange("(dc p) c -> p dc c", p=128))
            wv_sb = cp.tile([128, DC, 256], F32R, tag="wv")
            nc.sync.dma_start(wv_sb[:], wv_d.rearrange("(dc p) c -> p dc c", p=128))
            wout_sb = cp.tile([64, 4, D], F32R, tag="wout")
            nc.sync.dma_start(wout_sb[:], wout_d[:])

            # ---------- persistent state
            kT = pp.tile([128, 2, T], F32R, tag="kT")          # [qk-col, pair, t]
            v_sb = pp.tile([128, NT, 4, 65], BF16, tag="v")    # [t%128, tchunk, head, hd+one]
            outTe = pp.tile([65, 4, T], F32R, tag="oT", name="outTe")
            nc.vector.memset(v_sb[:, :, :, 64:65], 1.0)

            def body():
                with (
                    tc.tile_pool(name="work", bufs=1) as wp,
                    tc.tile_pool(name="work2", bufs=2) as wp2,
                    tc.tile_pool(name="work3", bufs=3) as wp3,
                    tc.tile_pool(name="pmisc", bufs=2, space="PSUM") as ps_m,
                    tc.tile_pool(name="pscore", bufs=2, space="PSUM") as ps_s,
                    tc.tile_pool(name="pout", bufs=1, space="PSUM") as ps_o,
                ):
                    for j in range(NJ):
                        t0 = 512 * j
                        # ---- load xT columns for rows [t0, t0+512)
                        #      (x is pre-transposed on the host)
                        xT = wp2.tile([128, DC, 512], F32R, tag="xT")
                        nc.sync.dma_start(
                            xT[:],
                            xt_d[:, t0 : t0 + 512].rearrange(
                                "(dc p) t -> p dc t", p=128
                            ),
                        )

                        # ---- project q,k for this tq tile (4 col-chunks)
                        qTj = wp2.tile([128, 2, 512], F32R, tag="qTj")
                        for cc in range(4):
                            pqk = ps_m.tile([128, 512], F32, tag="m")
                            for dc in range(DC):
                                nc.tensor.matmul(
                                    pqk[:],
                                    wqk_sb[:, dc, 128 * cc : 128 * (cc + 1)],
                                    xT[:, dc],
                                    start=(dc == 0),
                                    stop=(dc == DC - 1),
                                )
                            dst = qTj[:, cc] if cc < 2 else kT[:, cc - 2, t0 : t0 + 512]
                            nc.vector.tensor_scalar_add(dst, pqk[:], bqk_sb[:, cc : cc + 1])

                        # ---- project v for this tq tile
                        for ts in range(4):
                            pv = ps_m.tile([128, 512], F32, tag="m")
                            for dc in range(DC):
                                nc.tensor.matmul(
                                    pv[:, 0:256],
                                    xT[:, dc, 128 * ts : 128 * (ts + 1)],
                                    wv_sb[:, dc],
                                    start=(dc == 0),
                                    stop=(dc == DC - 1),
                                )
                            nc.vector.tensor_copy(
                                v_sb[:, 4 * j + ts, :, 0:64],
                                pv[:, 0:256].rearrange("p (h c) -> p h c", h=4),
                            )

                        # ---- causal attention for tq tile j, both head pairs
                        nchunk = 4 * (j + 1)
                        for hp in range(2 if not skip_attn else 0):
                            psO = ps_o.tile([128, 1024], F32, tag="po")
                            for i in range(nchunk):
                                dlt = 128 * i - 512 * j
                                dlt = dlt if dlt > 0 else 0
                                pS = ps_s.tile([128, 1024], F32, tag="ps")
                                for hh in range(2):
                                    nc.tensor.matmul(
                                        pS[:, 512 * hh + dlt : 512 * (hh + 1)],
                                        kT[64 * hh : 64 * (hh + 1), hp, 128 * i : 128 * (i + 1)],
                                        qTj[64 * hh : 64 * (hh + 1), hp, dlt:512],
                                        start=True,
                                        stop=True,
                                    )
                                pT = wp3.tile([128, 2, 512], BF16, tag="pT", bufs=3)
                                pSv = pS[:].rearrange("p (h w) -> p h w", h=2)
                                nc.scalar.activation(
                                    pT[:, :, dlt:512], pSv[:, :, dlt:512], AF.Exp, scale=SCALE
                                )
                                if i >= 4 * j:  # diagonal block: causal 0/1 mask
                                    for hh in range(2):
                                        nc.vector.tensor_tensor(
                                            pT[:, hh, dlt : dlt + 128],
                                            pT[:, hh, dlt : dlt + 128],
                                            tri[:],
                                            mybir.AluOpType.mult,
                                        )
                                for hh in range(2):
                                    nc.tensor.matmul(
                                        psO[0:65, 512 * hh + dlt : 512 * (hh + 1)],
                                        v_sb[:, i, 2 * hp + hh, :],
                                        pT[:, hh, dlt:512],
                                        start=(i == 0),
                                        stop=(i == nchunk - 1),
                                        skip_group_check=True,
                                    )
                            nc.vector.tensor_copy(
                                outTe[:, 2 * hp : 2 * hp + 2, t0 : t0 + 512],
                                psO[0:65, :].rearrange("p (h w) -> p h w", h=2),
                            )


                # ---- normalization tail: recip of sums rows, broadcast
                #      down partitions, scale outT in place (f32r out)
                with (
                    tc.tile_pool(name="norm", bufs=1) as npool,
                    tc.tile_pool(name="pbc", bufs=3, space="PSUM") as ps_b,
                ):
                    for h in range(4):
                        nc.sync.dma_start(
                            nrm_d[0, h : h + 1, :], outTe[64:65, h, :].bitcast(F32)
                        )
                    sT = npool.tile([128, 4, T // 128], F32, tag="sT")
                    nc.sync.dma_start(
                        sT[:], nrm_d[0, 0:4].rearrange("h (o p) -> p h o", p=128)
                    )
                    rT = npool.tile([128, 4, T // 128], F32, tag="rT")
                    nc.vector.reciprocal(rT[:], sT[:])
                    nc.sync.dma_start(
                        nrm_d[0, 4:8].rearrange("h (o p) -> p h o", p=128), rT[:]
                    )
                    for h in range(4):
                        rrow = npool.tile([1, T], F32R, tag="rrow", name=f"rrow{h}")
                        nc.sync.dma_start(
                            rrow[:], nrm_d[0, 4 + h : 5 + h, :].bitcast(F32R)
                        )
                        for q0 in range(0, T, 512):
                            pB = ps_b.tile([64, 512], F32, tag="pb")
                            nc.tensor.matmul(
                                pB[:], ones1r[:], rrow[:, q0 : q0 + 512],
                                start=True, stop=True,
                            )
                            nc.vector.tensor_tensor(
                                outTe[0:64, h, q0 : q0 + 512],
                                outTe[0:64, h, q0 : q0 + 512].bitcast(F32),
                                pB[:],
                                mybir.AluOpType.mult,
                            )

                # ---- out-projection tail (own psum pool; overlaps little
                #      but leaves the attention pipeline unperturbed)
                with (
                    tc.tile_pool(name="proj", bufs=3) as yp,
                    tc.tile_pool(name="py", bufs=2, space="PSUM") as ps_y,
                ):
                    for jt in range(NT):
                        for nh in range(2):
                            pY = ps_y.tile([128, 512], F32, tag="py")
                            for h in range(4):
                                nc.tensor.matmul(
                                    pY[:],
                                    outTe[0:64, h, 128 * jt : 128 * (jt + 1)],
                                    wout_sb[:, h, 512 * nh : 512 * (nh + 1)],
                                    start=(h == 0),
                                    stop=(h == 3),
                                )
                            y_sb = yp.tile([128, 512], F32, tag="y")
                            nc.scalar.copy(y_sb[:], pY[:])
                            nc.scalar.dma_start(
                                y_d[128 * jt : 128 * (jt + 1), 512 * nh : 512 * (nh + 1)],
                                y_sb[:],
                            )

            if reps == 1:
                body()
            else:
                with tc.For_i(0, reps, 1):
                    body()

    nc.compile()
    return nc


def shard_inputs(x, w_qkv, b_qkv, w_out, T):
    """Build the 8 per-core input maps (core c: batch c//4, head group c%4)."""
    x = np.asarray(x, dtype=np.float32)
    w_qkv = np.asarray(w_qkv, dtype=np.float32)
    b_qkv = np.asarray(b_qkv, dtype=np.float32)
    w_out = np.asarray(w_out, dtype=np.float32)
    in_maps = []
    for c in range(8):
        b, g = c // 4, c % 4
        qcols = slice(4 * g * 64, (4 * g + 4) * 64)
        kcols = slice(D + 4 * g * 64, D + (4 * g + 4) * 64)
        vcols = slice(2 * D + 4 * g * 64, 2 * D + (4 * g + 4) * 64)
        wqk = np.concatenate([w_qkv[:, qcols], w_qkv[:, kcols]], axis=1)  # [D, 512]
        wv = np.ascontiguousarray(w_qkv[:, vcols])  # [D, 256]
        bqk = np.concatenate([b_qkv[qcols], b_qkv[kcols]]).reshape(4, 128).T  # [128,4]
        wout = np.ascontiguousarray(
            w_out[256 * g : 256 * (g + 1), :].reshape(4, 64, D).transpose(1, 0, 2)
        )  # [64, 4, D]
        in_maps.append(
            {
                "xt": np.ascontiguousarray(x[b, :T].T),
                "wqk": np.ascontiguousarray(wqk),
                "wv": wv,
                "bqk": np.ascontiguousarray(bqk),
                "wout": wout,
            }
        )
    return in_maps


def assemble_output(results, b_qkv, b_out, w_out, T):
    b_qkv = np.asarray(b_qkv, dtype=np.float32)
    b_out = np.asarray(b_out, dtype=np.float32)
    w_out = np.asarray(w_out, dtype=np.float32)
    extra = b_out + b_qkv[2 * D :] @ w_out  # v-bias folds through softmax
    y = np.zeros((B, T, D), dtype=np.float32)
    for c in range(8):
        y[c // 4] += results[c]["y"]
    y += extra[None, None, :]
    return y


_cache = {}


def kernel(x, w_qkv, b_qkv, w_out, b_out):
    x = np.asarray(x, dtype=np.float32)
    T = x.shape[1]
    if T not in _cache:
        _cache[T] = build(T=T, reps=1)
    nc = _cache[T]
    in_maps = shard_inputs(x, w_qkv, b_qkv, w_out, T)
    for _attempt in range(3):
        res = run_bass_kernel_spmd(nc, in_maps, core_ids=list(range(8)), trace=False)
        y = assemble_output(res.results, b_qkv, b_out, w_out, T)
        if np.isfinite(y).all():  # guard against transient device flakes
            return y
    return y

